# revision 34
# baseline (speedup 1.0000x reference)
"""Multi-head attention (B=2, N=2048, D=1024, H=16) sharded over 8 trn2 cores.

Sharding: batch (2) x head-groups (4 groups of 4 heads) = 8 cores.
Each core computes, for its (batch b, head-group g):
  Q.T/K.T feature-major and V token-major projections of its group,
  S.T = K @ Q.T scores (keys on partitions, queries on free axis),
  P.T = exp(S.T / 8)  (no max subtraction -- scores are ~N(0,1), safe in fp32),
  ctx'.T = [V | ones].T @ P.T  (ones column yields softmax denominators),
  ctx.T normalized via gpsimd partition-broadcast of 1/sums,
  partial O = ctx.T.T @ w_o_g.T  (row-parallel O projection).
Host sums the 4 group partials per batch and adds b_o.

Schedule: input DMA order xq -> xk -> xv so Q/K projections finish first
(PSUM-chasing the arriving chunks with all 8 banks), scores+exp start
~29us while xv still streams; V' is built in the xv window using the
O-projection PSUM slots; ctx for chain c consumes exp outputs produced a
full chain earlier (pt pool buffers ~2 chains) so the PE never waits on
the ACT engine latency; softmax normalization runs on DVE+Pool only.

Matmul operands are bf16 (host pre-converts inputs; fp32 PSUM accumulation);
biases are applied in fp32 during PSUM eviction.
"""

import os
import sys

for _p in ("/opt/trn_rl_repo",):
    if _p not in sys.path and os.path.isdir(_p):
        sys.path.insert(0, _p)

import ml_dtypes
import numpy as np

import concourse.bass as bass
import concourse.tile as tile
from concourse import bacc, library_config, mybir
from concourse.bass_utils import run_bass_kernel_spmd

F32 = mybir.dt.float32
BF16 = mybir.dt.bfloat16
EXP = mybir.ActivationFunctionType.Exp

B = 2
D = 1024
N_HEADS = 16
DK = 64
N_CORES = 8
N_GROUPS = 4  # head groups (4 heads each) across cores within a batch
GF = D // N_GROUPS  # 256 features per group
HPG = N_HEADS // N_GROUPS  # 4 heads per group
PAIRS = HPG // 2  # head pairs (2 heads of 64 feats = 128 partitions)
KC = D // 128  # contraction chunks for the input projections


def build_nc(n_tok: int, loop_k: int = 1):
    """Build the single-core Bass program (same program for all 8 cores)."""
    import contextlib
    assert n_tok % 512 == 0
    QC = n_tok // 512  # query chunks of 512
    TT = n_tok // 128  # token (and key) tiles of 128

    nc = bacc.Bacc("TRN2", target_bir_lowering=False, debug=False,
                   num_devices=N_CORES)

    xqT = nc.dram_tensor("xqT", [D, n_tok], BF16, kind="ExternalInput")
    xkT = nc.dram_tensor("xkT", [D, n_tok], BF16, kind="ExternalInput")
    xvT = nc.dram_tensor("xvT", [D, n_tok], BF16, kind="ExternalInput")
    wqT = nc.dram_tensor("wqT", [D, GF], BF16, kind="ExternalInput")
    wkT = nc.dram_tensor("wkT", [D, GF], BF16, kind="ExternalInput")
    wvT = nc.dram_tensor("wvT", [D, GF], BF16, kind="ExternalInput")
    woT = nc.dram_tensor("woT", [GF, D], BF16, kind="ExternalInput")
    bq2 = nc.dram_tensor("bq2", [128, 2], F32, kind="ExternalInput")
    bk2 = nc.dram_tensor("bk2", [128, 2], F32, kind="ExternalInput")
    bvp = nc.dram_tensor("bvp", [1, GF], BF16, kind="ExternalInput")
    out_p = nc.dram_tensor("out_p", [n_tok, D], BF16, kind="ExternalOutput")
    # DRAM scratch rows for the softmax-reciprocal broadcast (DRAM source
    # APs may replicate with a 0-stride dim; SBUF partition APs may not)
    rscr = nc.dram_tensor("rscr", [4, 512], BF16, kind="Internal")

    chains = [(qc, p) for qc in range(QC) for p in range(PAIRS)]
    NCH = len(chains)
    KT2 = TT // 2  # kt2 groups per chain

    def mm(out, lhsT, rhs, **kw):
        nc.tensor.matmul(out, lhsT, rhs, **kw)

    with tile.TileContext(nc) as tc:
      with (tc.For_i(0, loop_k, 1) if loop_k > 1
            else contextlib.nullcontext()):
        with (
            tc.tile_pool(name="weights", bufs=1) as wpool,
            tc.tile_pool(name="acts", bufs=1) as apool,
            tc.tile_pool(name="xvs", bufs=1) as xvpool,
        ):
            wq_sb = wpool.tile([128, KC * GF], BF16, tag="wq")
            wk_sb = wpool.tile([128, KC * GF], BF16, tag="wk")
            wv_sb = wpool.tile([128, KC * GF], BF16, tag="wv")
            wo_sb = wpool.tile([128, 2 * D], BF16, tag="wo")
            bq_sb = wpool.tile([128, 2], F32, tag="bq")
            bk_sb = wpool.tile([128, 2], F32, tag="bk")
            bvp_sb = wpool.tile([1, GF], BF16, tag="bvp")

            # tiny first: biases, then the warm-up exp (ACT table load at
            # t~0, off the first-chain critical path)
            nc.sync.dma_start(bq_sb[:], bq2[:])
            nc.sync.dma_start(bk_sb[:], bk2[:])
            nc.sync.dma_start(bvp_sb[:], bvp[:])
            warm_in = wpool.tile([1, 8], F32, tag="warmi")
            warm_sb = wpool.tile([1, 8], F32, tag="warmo")
            nc.vector.memset(warm_in[:], 0.0)
            nc.scalar.activation(warm_sb[:], warm_in[:], EXP)
            ones_sb = wpool.tile([128, 128], BF16, tag="ones")
            nc.vector.memset(ones_sb[:], 1.0)
            # junk matmuls keep the PE continuously busy from t~0 so the
            # P-state ramp completes before the first real projection
            junk_sb = wpool.tile([128, 512], BF16, tag="junk")
            nc.vector.memset(junk_sb[:], 0.0)

            # Q.T / K.T feature-major [2 pair-tiles x 128, n_tok]
            qt_sb = apool.tile([128, PAIRS * n_tok], BF16, tag="qt")
            kt_sb = apool.tile([128, PAIRS * n_tok], BF16, tag="kt")
            # V' token-major with per-head ones column: [n_tok, HPG*65]
            v_sb = apool.tile([128, TT * HPG * 65], BF16, tag="v")
            nc.vector.memset(v_sb[:], 1.0)

            xv_t = [xvpool.tile([128, n_tok], BF16, tag=f"xv{k}",
                                name=f"xv{k}") for k in range(KC)]

            def load_w(w_dram, w_sb, halves=1):
                kh = KC // halves
                for h in range(halves):
                    nc.sync.dma_start(
                        w_sb[:].rearrange("p (k f) -> p k f", f=GF)
                        [:, h * kh:(h + 1) * kh, :],
                        w_dram[h * kh * 128:(h + 1) * kh * 128, :]
                        .rearrange("(k p) f -> p k f", p=128))

            with tc.tile_pool(name="xqk", bufs=1) as xqkpool:
                xq_t = [xqkpool.tile([128, n_tok], BF16, tag=f"xq{k}",
                                     name=f"xq{k}") for k in range(KC)]
                xk_t = [xqkpool.tile([128, n_tok], BF16, tag=f"xk{k}",
                                     name=f"xk{k}") for k in range(KC)]

                # arrival order = need order: wq, xq, wk, xk, wv, xv, wo
                # (Q chase runs while xk streams; K's pair-0 evictions land
                # right at the xk DMA tail, gating the first exp at ~30us)
                load_w(wqT, wq_sb, halves=2)
                for k in range(KC):
                    nc.sync.dma_start(xq_t[k][:],
                                      xqT[k * 128:(k + 1) * 128, :])
                load_w(wkT, wk_sb)
                for k in range(KC):
                    nc.sync.dma_start(xk_t[k][:],
                                      xkT[k * 128:(k + 1) * 128, :])
                load_w(wvT, wv_sb)
                for k in range(KC):
                    nc.sync.dma_start(xv_t[k][:],
                                      xvT[k * 128:(k + 1) * 128, :])
                nc.sync.dma_start(
                    wo_sb[:].rearrange("p (c f) -> p c f", f=D),
                    woT[:].rearrange("(c p) f -> p c f", p=128),
                )

                # ---- Q then K projections, PSUM-chasing the arriving x
                # chunks with all 8 banks; evictions interleaved into the
                # final contraction chunk ----
                with tc.tile_pool(name="pref", bufs=8,
                                  space="PSUM") as pref:
                    warm_ps = pref.tile([128, 512], F32, tag="pref",
                                        name="warmps")
                    for _ in range(14):
                        mm(warm_ps[:], junk_sb[:, 0:128], junk_sb[:],
                           start=True, stop=True)

                    def chase(xt, w_sb, b_sb, dst_sb, pfx, order):
                        tiles = {}
                        for (m, qq) in order:
                            tiles[(m, qq)] = pref.tile(
                                [128, 512], F32, tag="pref",
                                name=f"{pfx}{m}{qq}")
                        for k in range(KC):
                            last = (k == KC - 1)
                            for i, (m, qq) in enumerate(order):
                                mm(tiles[(m, qq)][:],
                                   w_sb[:, k * GF + m * 128:
                                        k * GF + (m + 1) * 128],
                                   xt[k][:, qq * 512:(qq + 1) * 512],
                                   start=(k == 0), stop=last)
                                if last:
                                    # evictions alternate DVE/ACT so they
                                    # drain in parallel with the last mms
                                    dst = dst_sb[:, m * n_tok + qq * 512:
                                                 m * n_tok + (qq + 1) * 512]
                                    if i % 2 == 0:
                                        nc.vector.tensor_scalar_add(
                                            dst, tiles[(m, qq)][:],
                                            b_sb[:, m:m + 1])
                                    else:
                                        nc.scalar.add(
                                            dst, tiles[(m, qq)][:],
                                            b_sb[:, m:m + 1])

                    # Q first (all evicted early); K second with pair-0
                    # tiles evicted first (chain 0 reads all of pair 0)
                    chase(xq_t, wq_sb, bq_sb, qt_sb, "q",
                          [(m, qq) for qq in range(QC)
                           for m in range(PAIRS)])
                    chase(xk_t, wk_sb, bk_sb, kt_sb, "k",
                          [(m, qq) for m in range(PAIRS)
                           for qq in range(QC)])

            # ---- attention: chain-offset pipeline ----
            with (
                tc.tile_pool(name="pt", bufs=40) as ptpool,
                tc.tile_pool(name="ctx", bufs=1) as ctxpool,
                tc.tile_pool(name="craw", bufs=4) as crpool,
                tc.tile_pool(name="rcb", bufs=2) as rcpool,
                tc.tile_pool(name="ost", bufs=4) as opool,
                tc.tile_pool(name="pss", bufs=2, space="PSUM") as pss,
            ):
                pt_map = {}
                cps_map = {}
                pools = {}
                ctx_t = {}
                for qc in range(QC):
                    for p in range(PAIRS):
                        ctx_t[(p, qc)] = ctxpool.tile(
                            [128, 512], BF16, tag=f"ctx{p}{qc}",
                            name=f"ctx{p}{qc}")
                oq = []
                ost_map = {}

                def scores_group(c, j):
                    qc, p = chains[c]
                    s2 = [pss.tile([128, 1024], F32, tag="s",
                                   name=f"s{c}_{j}_{h}") for h in range(2)]
                    for half in range(2):
                        kt = 2 * j + half
                        for h in range(2):
                            mm(s2[h][:, half * 512:(half + 1) * 512],
                               kt_sb[64 * h:64 * h + 64,
                                     p * n_tok + kt * 128:
                                     p * n_tok + (kt + 1) * 128],
                               qt_sb[64 * h:64 * h + 64,
                                     p * n_tok + qc * 512:
                                     p * n_tok + (qc + 1) * 512],
                               start=True, stop=True)
                    pts = []
                    for h in range(2):
                        ptile = ptpool.tile([128, 1024], BF16, tag="pt")
                        nc.scalar.activation(ptile[:], s2[h][:], EXP,
                                             scale=1.0 / np.sqrt(DK))
                        pts.append(ptile)
                    pt_map[(c, j)] = pts

                def ctx_group(c, j):
                    qc, p = chains[c]
                    if j == 0:
                        psctx = pools["psctx"]
                        cps_map[c] = [psctx.tile([128, 512], F32, tag="cps",
                                                 name=f"cps{c}{h}")
                                      for h in range(2)]
                    pts = pt_map.pop((c, j))
                    for half in range(2):
                        kt = 2 * j + half
                        for h in range(2):
                            hh = 2 * p + h
                            mm(cps_map[c][h][0:65, :],
                               v_sb[:, kt * HPG * 65 + hh * 65:
                                       kt * HPG * 65 + (hh + 1) * 65],
                               pts[h][:, half * 512:(half + 1) * 512],
                               start=(kt == 0), stop=(kt == TT - 1))

                def norm(c):
                    qc, p = chains[c]
                    cps = cps_map.pop(c)
                    last = (c == NCH - 1)
                    for h in range(2):
                        if last:
                            # tail: read PSUM directly, skip the
                            # slot-freeing copy (nothing follows)
                            src = cps[h]
                        else:
                            src = crpool.tile([128, 512], F32, tag="craw",
                                              name="craw")
                            nc.vector.tensor_copy(src[0:65, :],
                                                  cps[h][0:65, :])
                        rc = rcpool.tile([128, 512], BF16, tag="rc",
                                         name="rc")
                        with nc.allow_low_precision(
                                reason="bf16 recip -> bcast mm"):
                            nc.vector.reciprocal(rc[64:65, :],
                                                 src[64:65, :])
                        bb = rcpool.tile([128, 512], BF16, tag="bb",
                                         name="bb")
                        if last:
                            # tail: PE is idle; matmul broadcast avoids
                            # the DMA round-trip latency
                            bp = pools["psbo"].tile([128, 512], F32,
                                                    tag="bo", name="bp")
                            mm(bp[0:64, :], ones_sb[64:65, 0:64],
                               rc[64:65, :], start=True, stop=True)
                            nc.vector.tensor_copy(bb[0:64, :], bp[0:64, :])
                        else:
                            # mid-stream: PE is saturated; replicate the
                            # reciprocal row via a DRAM bounce on the idle
                            # SP queue instead of a PE matmul (the read-
                            # back uses a 0-stride DRAM source dim)
                            row = rscr[(2 * c + h) % 4:
                                       (2 * c + h) % 4 + 1, :]
                            nc.sync.dma_start(row, rc[64:65, :])
                            brd = bass.AP(row.tensor, row.offset,
                                          [[0, 64]] + list(row.ap[1:]))
                            nc.sync.dma_start(bb[0:64, :], brd)
                        nc.vector.tensor_mul(
                            ctx_t[(p, qc)][64 * h:64 * h + 64, :],
                            src[0:64, :], bb[0:64, :])
                    if p == PAIRS - 1:
                        oq.extend((qc, s, n) for s in range(4)
                                  for n in range(2))

                def emit_o_unit():
                    qc, s, n = oq.pop(0)
                    key = (qc, s)
                    if n == 0:
                        ost_map[key] = opool.tile([128, D], BF16, tag="ost",
                                                  name="ost")
                    op = pools["psbo"].tile([128, 512], F32, tag="bo",
                                            name="op")
                    for cp in range(PAIRS):
                        mm(op[:],
                           ctx_t[(cp, qc)][:, s * 128:(s + 1) * 128],
                           wo_sb[:, cp * D + n * 512:cp * D + (n + 1) * 512],
                           start=(cp == 0), stop=(cp == PAIRS - 1))
                    dst = ost_map[key][:, n * 512:(n + 1) * 512]
                    if qc == QC - 1:
                        # tail: ACT is idle; alternate engines and ship each
                        # half as soon as it lands so the drain overlaps
                        if (s + n) % 2 == 0:
                            nc.scalar.copy(dst, op[:])
                        else:
                            nc.vector.tensor_copy(dst, op[:])
                    else:
                        nc.vector.tensor_copy(dst, op[:])
                    nc.sync.dma_start(
                        out_p[qc * 512 + s * 128:qc * 512 + (s + 1) * 128,
                              n * 512:(n + 1) * 512],
                        dst)
                    if n == 1:
                        ost_map.pop(key)

                # ---- V' construction: rounds of 4 token-tiles, one
                # accumulation region per PSUM bank (4 banks from the
                # nested pssB pool, which closes before psctx/psbo open) --
                NR = TT // 4  # 4 rounds
                vslots = {}

                def v_mms(r, k):
                    if k == 0:
                        vslots[r] = [pools["pssB"].tile(
                            [128, 512], F32, tag="vp", name=f"vsl{r}{i}")
                            for i in range(4)]
                        for q in range(4):
                            mm(vslots[r][q][:, 0:256],
                               ones_sb[0:1, 0:128], bvp_sb[0:1, :],
                               start=True, stop=False)
                    for q in range(4):
                        tt = 4 * r + q
                        reg = vslots[r][q][:, 0:256]
                        mm(reg,
                           xv_t[k][:, tt * 128:(tt + 1) * 128],
                           wv_sb[:, k * GF:(k + 1) * GF],
                           start=False, stop=(k == KC - 1))
                    if k == KC - 1:
                        for q in range(4):
                            tt = 4 * r + q
                            reg = vslots[r][q][:, 0:256]
                            dst = v_sb[:, tt * HPG * 65:(tt + 1) * HPG * 65]
                            nc.vector.tensor_copy(
                                dst.rearrange("p (h c) -> p h c",
                                              c=65)[:, :, 0:DK],
                                reg.rearrange("p (h c) -> p h c", c=DK))
                        del vslots[r]

                with tc.tile_pool(name="pssB", bufs=4,
                                  space="PSUM") as pssB:
                    pools["pssB"] = pssB
                    # scores chain 0, V' round 0 arrival-aligned: the
                    # exp-paced score emission stays ahead of the xv
                    # arrivals, so v-unit (0, j) never blocks the queue
                    for j in range(KT2):
                        scores_group(0, j)
                        v_mms(0, j)
                    # scores chain 1 with the remaining V' rounds packed
                    # into the ACT-paced slack (all xv chunks resident)
                    vq = [(r, k) for r in range(1, NR) for k in range(KC)]
                    for j in range(KT2):
                        scores_group(1, j)
                        for _ in range(3):
                            if vq:
                                v_mms(*vq.pop(0))

                with (
                    tc.tile_pool(name="psctx", bufs=2,
                                 space="PSUM") as psctx,
                    tc.tile_pool(name="psbo", bufs=2, space="PSUM") as psbo,
                ):
                    pools["psctx"] = psctx
                    pools["psbo"] = psbo
                    # drain the two buffered chains while ACT chews the
                    # freshly emitted scores of chains 2 and 3
                    for j in range(KT2):
                        scores_group(2, j)
                        ctx_group(0, j)
                    norm(0)
                    for j in range(KT2):
                        scores_group(3, j)
                        ctx_group(1, j)
                    norm(1)
                    # remaining scores emitted from a global queue: the
                    # ctx-only re-convergence loop takes them at half rate
                    # so ACT never starves at the chain-4 boundary
                    sq = [(cc, jj) for cc in range(4, NCH)
                          for jj in range(KT2)]
                    for j in range(KT2):
                        if j % 2 == 0 and sq:
                            scores_group(*sq.pop(0))
                        ctx_group(2, j)
                        for _ in range(2):
                            if oq:
                                emit_o_unit()
                    norm(2)
                    for c in range(3, NCH):
                        for j in range(KT2):
                            if sq:
                                scores_group(*sq.pop(0))
                            ctx_group(c, j)
                            if oq:
                                emit_o_unit()
                        norm(c)
                    while oq:
                        emit_o_unit()

    nc.compile()
    return nc


_NC_CACHE: dict[int, object] = {}


def get_nc(n_tok: int):
    if n_tok not in _NC_CACHE:
        _NC_CACHE[n_tok] = build_nc(n_tok)
    return _NC_CACHE[n_tok]


def make_in_maps(query, key, value, w_q, b_q, w_k, b_k, w_v, b_v, w_o, b_o):
    n_tok = query.shape[1]
    bf16 = ml_dtypes.bfloat16
    xT = {}
    for b in range(B):
        xT[("q", b)] = np.ascontiguousarray(query[b].T.astype(bf16))
        xT[("k", b)] = np.ascontiguousarray(key[b].T.astype(bf16))
        xT[("v", b)] = np.ascontiguousarray(value[b].T.astype(bf16))
    in_maps = []
    for core in range(N_CORES):
        b, g = divmod(core, N_GROUPS)
        gs = slice(g * GF, (g + 1) * GF)
        in_maps.append({
            "xqT": xT[("q", b)],
            "xkT": xT[("k", b)],
            "xvT": xT[("v", b)],
            "wqT": np.ascontiguousarray(w_q[gs, :].T.astype(bf16)),
            "wkT": np.ascontiguousarray(w_k[gs, :].T.astype(bf16)),
            "wvT": np.ascontiguousarray(w_v[gs, :].T.astype(bf16)),
            "woT": np.ascontiguousarray(w_o[:, gs].T.astype(bf16)),
            "bq2": np.ascontiguousarray(
                b_q[gs].reshape(2, 128).T, np.float32),
            "bk2": np.ascontiguousarray(
                b_k[gs].reshape(2, 128).T, np.float32),
            "bvp": np.ascontiguousarray(b_v[gs].reshape(1, GF).astype(bf16)),
        })
    return in_maps


def kernel(**inputs):
    query = np.asarray(inputs["query"], np.float32)
    n_tok = query.shape[1]
    nc = get_nc(n_tok)
    in_maps = make_in_maps(
        query, np.asarray(inputs["key"], np.float32),
        np.asarray(inputs["value"], np.float32),
        np.asarray(inputs["w_q"], np.float32), np.asarray(inputs["b_q"], np.float32),
        np.asarray(inputs["w_k"], np.float32), np.asarray(inputs["b_k"], np.float32),
        np.asarray(inputs["w_v"], np.float32), np.asarray(inputs["b_v"], np.float32),
        np.asarray(inputs["w_o"], np.float32), np.asarray(inputs["b_o"], np.float32),
    )
    res = run_bass_kernel_spmd(nc, in_maps, core_ids=list(range(N_CORES)))
    out = np.zeros((B, n_tok, D), np.float32)
    for core in range(N_CORES):
        b = core // N_GROUPS
        out[b] += res.results[core]["out_p"].astype(np.float32)
    out += np.asarray(inputs["b_o"], np.float32)
    return out


# revision 39
# speedup vs baseline: 1.1231x; 1.1231x over previous
"""Multi-head attention (B=2, N=2048, D=1024, H=16) sharded over 8 trn2 cores.

Sharding: batch (2) x head-groups (4 groups of 4 heads) = 8 cores.
Each core computes, for its (batch b, head-group g):
  Q.T/K.T feature-major and V token-major projections of its group,
  S.T = K @ Q.T scores (keys on partitions, queries on free axis),
  P.T = exp(S.T / 8)  (no max subtraction -- scores are ~N(0,1), safe in fp32),
  ctx'.T = [V | ones].T @ P.T  (ones column yields softmax denominators),
  ctx.T normalized via gpsimd partition-broadcast of 1/sums,
  partial O = ctx.T.T @ w_o_g.T  (row-parallel O projection).
Host sums the 4 group partials per batch and adds b_o.

Schedule: input DMA order xq -> xk -> xv so Q/K projections finish first
(PSUM-chasing the arriving chunks with all 8 banks), scores+exp start
~29us while xv still streams; V' is built in the xv window using the
O-projection PSUM slots; ctx for chain c consumes exp outputs produced a
full chain earlier (pt pool buffers ~2 chains) so the PE never waits on
the ACT engine latency; softmax normalization runs on DVE+Pool only.

Matmul operands are bf16 (host pre-converts inputs; fp32 PSUM accumulation);
biases are applied in fp32 during PSUM eviction.
"""

import os
import sys

for _p in ("/opt/trn_rl_repo",):
    if _p not in sys.path and os.path.isdir(_p):
        sys.path.insert(0, _p)

import ml_dtypes
import numpy as np

import concourse.bass as bass
import concourse.tile as tile
from concourse import bacc, mybir
from concourse.bass_utils import run_bass_kernel_spmd

F32 = mybir.dt.float32
BF16 = mybir.dt.bfloat16
EXP = mybir.ActivationFunctionType.Exp

B = 2
D = 1024
N_HEADS = 16
DK = 64
N_CORES = 8
N_GROUPS = 4  # head groups (4 heads each) across cores within a batch
GF = D // N_GROUPS  # 256 features per group
HPG = N_HEADS // N_GROUPS  # 4 heads per group
PAIRS = HPG // 2  # head pairs (2 heads of 64 feats = 128 partitions)
KC = D // 128  # contraction chunks for the input projections


def build_nc(n_tok: int, loop_k: int = 1):
    """Build the single-core Bass program (same program for all 8 cores)."""
    import contextlib
    assert n_tok % 512 == 0
    QC = n_tok // 512  # query chunks of 512
    TT = n_tok // 128  # token (and key) tiles of 128

    nc = bacc.Bacc("TRN2", target_bir_lowering=False, debug=False,
                   num_devices=N_CORES)

    xqT = nc.dram_tensor("xqT", [D, n_tok], BF16, kind="ExternalInput")
    xkT = nc.dram_tensor("xkT", [D, n_tok], BF16, kind="ExternalInput")
    xvT = nc.dram_tensor("xvT", [D, n_tok], BF16, kind="ExternalInput")
    wqT = nc.dram_tensor("wqT", [D, GF], BF16, kind="ExternalInput")
    wkT = nc.dram_tensor("wkT", [D, GF], BF16, kind="ExternalInput")
    wvT = nc.dram_tensor("wvT", [D, GF], BF16, kind="ExternalInput")
    woT = nc.dram_tensor("woT", [GF, D], BF16, kind="ExternalInput")
    bq2 = nc.dram_tensor("bq2", [128, 2], F32, kind="ExternalInput")
    bk2 = nc.dram_tensor("bk2", [128, 2], F32, kind="ExternalInput")
    bvp = nc.dram_tensor("bvp", [1, GF], BF16, kind="ExternalInput")
    out_p = nc.dram_tensor("out_p", [n_tok, D], BF16, kind="ExternalOutput")
    # DRAM scratch rows for the softmax-reciprocal broadcast (DRAM source
    # APs may replicate with a 0-stride dim; SBUF partition APs may not)
    rscr = nc.dram_tensor("rscr", [4, 512], BF16, kind="Internal")

    chains = [(qc, p) for qc in range(QC) for p in range(PAIRS)]
    NCH = len(chains)
    KT2 = TT // 2  # kt2 groups per chain

    def mm(out, lhsT, rhs, **kw):
        nc.tensor.matmul(out, lhsT, rhs, **kw)

    with tile.TileContext(nc) as tc:
      with (tc.For_i(0, loop_k, 1) if loop_k > 1
            else contextlib.nullcontext()):
        with (
            tc.tile_pool(name="weights", bufs=1) as wpool,
            tc.tile_pool(name="acts", bufs=1) as apool,
            tc.tile_pool(name="xvs", bufs=1) as xvpool,
        ):
            wq_sb = wpool.tile([128, KC * GF], BF16, tag="wq")
            wk_sb = wpool.tile([128, KC * GF], BF16, tag="wk")
            wv_sb = wpool.tile([128, KC * GF], BF16, tag="wv")
            wo_sb = wpool.tile([128, 2 * D], BF16, tag="wo")
            bq_sb = wpool.tile([128, 2], F32, tag="bq")
            bk_sb = wpool.tile([128, 2], F32, tag="bk")
            bvp_sb = wpool.tile([1, GF], BF16, tag="bvp")

            # tiny bias DMAs go out on the ACT queue (also HWDGE) so their
            # DGE setup time does not delay the SP x-stream; warm-up exp
            # loads the ACT table at t~0, off the first-chain critical path
            nc.scalar.dma_start(bq_sb[:], bq2[:])
            nc.scalar.dma_start(bk_sb[:], bk2[:])
            nc.scalar.dma_start(bvp_sb[:], bvp[:])
            warm_in = wpool.tile([1, 8], F32, tag="warmi")
            warm_sb = wpool.tile([1, 8], F32, tag="warmo")
            nc.vector.memset(warm_in[:], 0.0)
            nc.scalar.activation(warm_sb[:], warm_in[:], EXP)
            ones_sb = wpool.tile([128, 128], BF16, tag="ones")
            nc.vector.memset(ones_sb[:], 1.0)
            # junk matmuls keep the PE continuously busy from t~0 so the
            # P-state ramp completes before the first real projection
            junk_sb = wpool.tile([128, 512], BF16, tag="junk")
            nc.vector.memset(junk_sb[:], 0.0)

            # Q.T / K.T feature-major [2 pair-tiles x 128, n_tok]
            qt_sb = apool.tile([128, PAIRS * n_tok], BF16, tag="qt")
            kt_sb = apool.tile([128, PAIRS * n_tok], BF16, tag="kt")
            # V' token-major with per-head ones column: [n_tok, HPG*65]
            v_sb = apool.tile([128, TT * HPG * 65], BF16, tag="v")
            nc.vector.memset(v_sb[:], 1.0)

            xv_t = [xvpool.tile([128, n_tok], BF16, tag=f"xv{k}",
                                name=f"xv{k}") for k in range(KC)]

            def load_w(w_dram, w_sb, halves=1):
                kh = KC // halves
                for h in range(halves):
                    nc.sync.dma_start(
                        w_sb[:].rearrange("p (k f) -> p k f", f=GF)
                        [:, h * kh:(h + 1) * kh, :],
                        w_dram[h * kh * 128:(h + 1) * kh * 128, :]
                        .rearrange("(k p) f -> p k f", p=128))

            with tc.tile_pool(name="xqk", bufs=1) as xqkpool:
                xq_t = [xqkpool.tile([128, n_tok], BF16, tag=f"xq{k}",
                                     name=f"xq{k}") for k in range(KC)]
                xk_t = [xqkpool.tile([128, n_tok], BF16, tag=f"xk{k}",
                                     name=f"xk{k}") for k in range(KC)]

                # arrival order = need order: wq, xq, wk, xk, wv, xv, wo
                # (Q chase runs while xk streams; K's pair-0 evictions land
                # right at the xk DMA tail, gating the first exp at ~30us)
                load_w(wqT, wq_sb, halves=2)
                for k in range(KC):
                    nc.sync.dma_start(xq_t[k][:],
                                      xqT[k * 128:(k + 1) * 128, :])
                load_w(wkT, wk_sb)
                for k in range(KC):
                    nc.sync.dma_start(xk_t[k][:],
                                      xkT[k * 128:(k + 1) * 128, :])
                load_w(wvT, wv_sb)
                for k in range(KC):
                    nc.sync.dma_start(xv_t[k][:],
                                      xvT[k * 128:(k + 1) * 128, :])
                nc.sync.dma_start(
                    wo_sb[:].rearrange("p (c f) -> p c f", f=D),
                    woT[:].rearrange("(c p) f -> p c f", p=128),
                )

                # ---- Q then K projections, PSUM-chasing the arriving x
                # chunks with all 8 banks; evictions interleaved into the
                # final contraction chunk ----
                with tc.tile_pool(name="pref", bufs=8,
                                  space="PSUM") as pref:
                    warm_ps = pref.tile([128, 512], F32, tag="pref",
                                        name="warmps")
                    for _ in range(10):
                        mm(warm_ps[:], junk_sb[:, 0:128], junk_sb[:],
                           start=True, stop=True)

                    def chase(xt, w_sb, b_sb, dst_sb, pfx, order):
                        tiles = {}
                        for (m, qq) in order:
                            tiles[(m, qq)] = pref.tile(
                                [128, 512], F32, tag="pref",
                                name=f"{pfx}{m}{qq}")
                        for k in range(KC):
                            last = (k == KC - 1)
                            for i, (m, qq) in enumerate(order):
                                mm(tiles[(m, qq)][:],
                                   w_sb[:, k * GF + m * 128:
                                        k * GF + (m + 1) * 128],
                                   xt[k][:, qq * 512:(qq + 1) * 512],
                                   start=(k == 0), stop=last)
                                if last:
                                    # evictions alternate DVE/ACT so they
                                    # drain in parallel with the last mms
                                    dst = dst_sb[:, m * n_tok + qq * 512:
                                                 m * n_tok + (qq + 1) * 512]
                                    if i % 2 == 0:
                                        nc.vector.tensor_scalar_add(
                                            dst, tiles[(m, qq)][:],
                                            b_sb[:, m:m + 1])
                                    else:
                                        nc.scalar.add(
                                            dst, tiles[(m, qq)][:],
                                            b_sb[:, m:m + 1])

                    # Q first (all evicted early); K second with pair-0
                    # tiles evicted first (chain 0 reads all of pair 0)
                    chase(xq_t, wq_sb, bq_sb, qt_sb, "q",
                          [(m, qq) for qq in range(QC)
                           for m in range(PAIRS)])
                    chase(xk_t, wk_sb, bk_sb, kt_sb, "k",
                          [(m, qq) for m in range(PAIRS)
                           for qq in range(QC)])

            # ---- attention: chain-offset pipeline ----
            with (
                tc.tile_pool(name="pt", bufs=40) as ptpool,
                tc.tile_pool(name="ctx", bufs=1) as ctxpool,
                tc.tile_pool(name="craw", bufs=4) as crpool,
                tc.tile_pool(name="rcb", bufs=2) as rcpool,
                tc.tile_pool(name="ost", bufs=4) as opool,
                tc.tile_pool(name="pss", bufs=2, space="PSUM") as pss,
            ):
                pt_map = {}
                cps_map = {}
                pools = {}
                ctx_t = {}
                for qc in range(QC):
                    for p in range(PAIRS):
                        ctx_t[(p, qc)] = ctxpool.tile(
                            [128, 512], BF16, tag=f"ctx{p}{qc}",
                            name=f"ctx{p}{qc}")
                oq = []
                ost_map = {}

                def scores_group(c, j):
                    qc, p = chains[c]
                    s2 = [pss.tile([128, 1024], F32, tag="s",
                                   name=f"s{c}_{j}_{h}") for h in range(2)]
                    for half in range(2):
                        kt = 2 * j + half
                        for h in range(2):
                            mm(s2[h][:, half * 512:(half + 1) * 512],
                               kt_sb[64 * h:64 * h + 64,
                                     p * n_tok + kt * 128:
                                     p * n_tok + (kt + 1) * 128],
                               qt_sb[64 * h:64 * h + 64,
                                     p * n_tok + qc * 512:
                                     p * n_tok + (qc + 1) * 512],
                               start=True, stop=True)
                    pts = []
                    for h in range(2):
                        ptile = ptpool.tile([128, 1024], BF16, tag="pt")
                        nc.scalar.activation(ptile[:], s2[h][:], EXP,
                                             scale=1.0 / np.sqrt(DK))
                        pts.append(ptile)
                    pt_map[(c, j)] = pts

                def ctx_group(c, j):
                    qc, p = chains[c]
                    if j == 0:
                        psctx = pools["psctx"]
                        cps_map[c] = [psctx.tile([128, 512], F32, tag="cps",
                                                 name=f"cps{c}{h}")
                                      for h in range(2)]
                    pts = pt_map.pop((c, j))
                    for half in range(2):
                        kt = 2 * j + half
                        for h in range(2):
                            hh = 2 * p + h
                            mm(cps_map[c][h][0:65, :],
                               v_sb[:, kt * HPG * 65 + hh * 65:
                                       kt * HPG * 65 + (hh + 1) * 65],
                               pts[h][:, half * 512:(half + 1) * 512],
                               start=(kt == 0), stop=(kt == TT - 1))

                def norm(c):
                    qc, p = chains[c]
                    cps = cps_map.pop(c)
                    last = (c == NCH - 1)
                    for h in range(2):
                        if last:
                            # tail: read PSUM directly, skip the
                            # slot-freeing copy (nothing follows)
                            src = cps[h]
                        else:
                            src = crpool.tile([128, 512], F32, tag="craw",
                                              name="craw")
                            nc.vector.tensor_copy(src[0:65, :],
                                                  cps[h][0:65, :])
                        rc = rcpool.tile([128, 512], BF16, tag="rc",
                                         name="rc")
                        with nc.allow_low_precision(
                                reason="bf16 recip -> bcast mm"):
                            nc.vector.reciprocal(rc[64:65, :],
                                                 src[64:65, :])
                        bb = rcpool.tile([128, 512], BF16, tag="bb",
                                         name="bb")
                        if last:
                            # tail: PE is idle; matmul broadcast avoids
                            # the DMA round-trip latency
                            bp = pools["psbo"].tile([128, 512], F32,
                                                    tag="bo", name="bp")
                            mm(bp[0:64, :], ones_sb[64:65, 0:64],
                               rc[64:65, :], start=True, stop=True)
                            nc.vector.tensor_copy(bb[0:64, :], bp[0:64, :])
                        else:
                            # mid-stream: PE is saturated; replicate the
                            # reciprocal row via a DRAM bounce on the idle
                            # SP queue instead of a PE matmul (the read-
                            # back uses a 0-stride DRAM source dim)
                            row = rscr[(2 * c + h) % 4:
                                       (2 * c + h) % 4 + 1, :]
                            nc.sync.dma_start(row, rc[64:65, :])
                            brd = bass.AP(row.tensor, row.offset,
                                          [[0, 64]] + list(row.ap[1:]))
                            nc.sync.dma_start(bb[0:64, :], brd)
                        nc.vector.tensor_mul(
                            ctx_t[(p, qc)][64 * h:64 * h + 64, :],
                            src[0:64, :], bb[0:64, :])
                    if p == PAIRS - 1:
                        oq.extend((qc, s, n) for s in range(4)
                                  for n in range(2))

                ocnt = [0]

                def emit_o_unit():
                    qc, s, n = oq.pop(0)
                    key = (qc, s)
                    if n == 0:
                        ost_map[key] = opool.tile([128, D], BF16, tag="ost",
                                                  name="ost")
                    ocnt[0] += 1
                    if qc == QC - 1 and ocnt[0] % 2 == 0:
                        # tail drain: the scores PSUM banks are free after
                        # the last exp — borrow them so the O projection
                        # rotates through 4 slots instead of 2
                        op = pss.tile([128, 1024], F32, tag="s",
                                      name="opt")[:, 0:512]
                    else:
                        op = pools["psbo"].tile([128, 512], F32, tag="bo",
                                                name="op")[:, 0:512]
                    for cp in range(PAIRS):
                        mm(op,
                           ctx_t[(cp, qc)][:, s * 128:(s + 1) * 128],
                           wo_sb[:, cp * D + n * 512:cp * D + (n + 1) * 512],
                           start=(cp == 0), stop=(cp == PAIRS - 1))
                    dst = ost_map[key][:, n * 512:(n + 1) * 512]
                    if qc == QC - 1:
                        # tail: ACT is idle; alternate engines and ship each
                        # half as soon as it lands so the drain overlaps
                        if (s + n) % 2 == 0:
                            nc.scalar.copy(dst, op)
                        else:
                            nc.vector.tensor_copy(dst, op)
                    else:
                        nc.vector.tensor_copy(dst, op)
                    nc.sync.dma_start(
                        out_p[qc * 512 + s * 128:qc * 512 + (s + 1) * 128,
                              n * 512:(n + 1) * 512],
                        dst)
                    if n == 1:
                        ost_map.pop(key)

                # ---- V' construction: rounds of 4 token-tiles, one
                # accumulation region per PSUM bank (4 banks from the
                # nested pssB pool, which closes before psctx/psbo open) --
                NR = TT // 4  # 4 rounds
                vslots = {}

                def v_mms(r, k):
                    if k == 0:
                        vslots[r] = [pools["pssB"].tile(
                            [128, 512], F32, tag="vp", name=f"vsl{r}{i}")
                            for i in range(4)]
                        for q in range(4):
                            mm(vslots[r][q][:, 0:256],
                               ones_sb[0:1, 0:128], bvp_sb[0:1, :],
                               start=True, stop=False)
                    for q in range(4):
                        tt = 4 * r + q
                        reg = vslots[r][q][:, 0:256]
                        mm(reg,
                           xv_t[k][:, tt * 128:(tt + 1) * 128],
                           wv_sb[:, k * GF:(k + 1) * GF],
                           start=False, stop=(k == KC - 1))
                    if k == KC - 1:
                        for q in range(4):
                            tt = 4 * r + q
                            reg = vslots[r][q][:, 0:256]
                            dst = v_sb[:, tt * HPG * 65:(tt + 1) * HPG * 65]
                            nc.vector.tensor_copy(
                                dst.rearrange("p (h c) -> p h c",
                                              c=65)[:, :, 0:DK],
                                reg.rearrange("p (h c) -> p h c", c=DK))
                        del vslots[r]

                with tc.tile_pool(name="pssB", bufs=4,
                                  space="PSUM") as pssB:
                    pools["pssB"] = pssB
                    # scores chain 0, V' round 0 arrival-aligned: the
                    # exp-paced score emission stays ahead of the xv
                    # arrivals, so v-unit (0, j) never blocks the queue
                    for j in range(KT2):
                        scores_group(0, j)
                        v_mms(0, j)
                    # scores chain 1 with the remaining V' rounds packed
                    # into the ACT-paced slack (all xv chunks resident)
                    vq = [(r, k) for r in range(1, NR) for k in range(KC)]
                    for j in range(KT2):
                        scores_group(1, j)
                        for _ in range(3):
                            if vq:
                                v_mms(*vq.pop(0))

                with (
                    tc.tile_pool(name="psctx", bufs=2,
                                 space="PSUM") as psctx,
                    tc.tile_pool(name="psbo", bufs=2, space="PSUM") as psbo,
                ):
                    pools["psctx"] = psctx
                    pools["psbo"] = psbo
                    # drain the two buffered chains while ACT chews the
                    # freshly emitted scores of chains 2 and 3
                    for j in range(KT2):
                        scores_group(2, j)
                        ctx_group(0, j)
                    norm(0)
                    for j in range(KT2):
                        scores_group(3, j)
                        ctx_group(1, j)
                    norm(1)
                    # remaining scores emitted from a global queue: the
                    # ctx-only re-convergence loop takes them at half rate
                    # so ACT never starves at the chain-4 boundary
                    sq = [(cc, jj) for cc in range(4, NCH)
                          for jj in range(KT2)]
                    for j in range(KT2):
                        if j % 2 == 0 and sq:
                            scores_group(*sq.pop(0))
                        ctx_group(2, j)
                        for _ in range(2):
                            if oq:
                                emit_o_unit()
                    norm(2)
                    for c in range(3, NCH):
                        for j in range(KT2):
                            if sq:
                                scores_group(*sq.pop(0))
                            ctx_group(c, j)
                            if oq:
                                emit_o_unit()
                        norm(c)
                    while oq:
                        emit_o_unit()

    nc.compile()
    return nc


_NC_CACHE: dict[int, object] = {}


def get_nc(n_tok: int):
    if n_tok not in _NC_CACHE:
        _NC_CACHE[n_tok] = build_nc(n_tok)
    return _NC_CACHE[n_tok]


def make_in_maps(query, key, value, w_q, b_q, w_k, b_k, w_v, b_v, w_o, b_o):
    n_tok = query.shape[1]
    bf16 = ml_dtypes.bfloat16
    xT = {}
    for b in range(B):
        xT[("q", b)] = np.ascontiguousarray(query[b].T.astype(bf16))
        xT[("k", b)] = np.ascontiguousarray(key[b].T.astype(bf16))
        xT[("v", b)] = np.ascontiguousarray(value[b].T.astype(bf16))
    in_maps = []
    for core in range(N_CORES):
        b, g = divmod(core, N_GROUPS)
        gs = slice(g * GF, (g + 1) * GF)
        in_maps.append({
            "xqT": xT[("q", b)],
            "xkT": xT[("k", b)],
            "xvT": xT[("v", b)],
            "wqT": np.ascontiguousarray(w_q[gs, :].T.astype(bf16)),
            "wkT": np.ascontiguousarray(w_k[gs, :].T.astype(bf16)),
            "wvT": np.ascontiguousarray(w_v[gs, :].T.astype(bf16)),
            "woT": np.ascontiguousarray(w_o[:, gs].T.astype(bf16)),
            "bq2": np.ascontiguousarray(
                b_q[gs].reshape(2, 128).T, np.float32),
            "bk2": np.ascontiguousarray(
                b_k[gs].reshape(2, 128).T, np.float32),
            "bvp": np.ascontiguousarray(b_v[gs].reshape(1, GF).astype(bf16)),
        })
    return in_maps


def kernel(**inputs):
    query = np.asarray(inputs["query"], np.float32)
    n_tok = query.shape[1]
    nc = get_nc(n_tok)
    in_maps = make_in_maps(
        query, np.asarray(inputs["key"], np.float32),
        np.asarray(inputs["value"], np.float32),
        np.asarray(inputs["w_q"], np.float32), np.asarray(inputs["b_q"], np.float32),
        np.asarray(inputs["w_k"], np.float32), np.asarray(inputs["b_k"], np.float32),
        np.asarray(inputs["w_v"], np.float32), np.asarray(inputs["b_v"], np.float32),
        np.asarray(inputs["w_o"], np.float32), np.asarray(inputs["b_o"], np.float32),
    )
    res = run_bass_kernel_spmd(nc, in_maps, core_ids=list(range(N_CORES)))
    out = np.zeros((B, n_tok, D), np.float32)
    for core in range(N_CORES):
        b = core // N_GROUPS
        out[b] += res.results[core]["out_p"].astype(np.float32)
    out += np.asarray(inputs["b_o"], np.float32)
    return out


# revision 42
# speedup vs baseline: 1.2139x; 1.0809x over previous
"""Multi-head attention (B=2, N=2048, D=1024, H=16) sharded over 8 trn2 cores.

Sharding: batch (2) x head-groups (4 groups of 4 heads) = 8 cores.
Each core computes, for its (batch b, head-group g):
  Q.T/K.T feature-major and V token-major projections of its group,
  S.T = K @ Q.T scores (keys on partitions, queries on free axis),
  P.T = exp(S.T / 8)  (no max subtraction -- scores are ~N(0,1), safe in fp32),
  ctx'.T = [V | ones].T @ P.T  (ones column yields softmax denominators),
  ctx.T normalized via gpsimd partition-broadcast of 1/sums,
  partial O = ctx.T.T @ w_o_g.T  (row-parallel O projection).
Host sums the 4 group partials per batch and adds b_o.

Schedule: input DMA order xq -> xk -> xv so Q/K projections finish first
(PSUM-chasing the arriving chunks with all 8 banks), scores+exp start
~29us while xv still streams; V' is built in the xv window using the
O-projection PSUM slots; ctx for chain c consumes exp outputs produced a
full chain earlier (pt pool buffers ~2 chains) so the PE never waits on
the ACT engine latency; softmax normalization runs on DVE+Pool only.

Matmul operands are bf16 (host pre-converts inputs; fp32 PSUM accumulation);
biases are applied in fp32 during PSUM eviction.
"""

import os
import sys

for _p in ("/opt/trn_rl_repo",):
    if _p not in sys.path and os.path.isdir(_p):
        sys.path.insert(0, _p)

import ml_dtypes
import numpy as np

import concourse.bass as bass
import concourse.tile as tile
from concourse import bacc, mybir
from concourse.bass_utils import run_bass_kernel_spmd

F32 = mybir.dt.float32
BF16 = mybir.dt.bfloat16
EXP = mybir.ActivationFunctionType.Exp

B = 2
D = 1024
N_HEADS = 16
DK = 64
N_CORES = 8
N_GROUPS = 4  # head groups (4 heads each) across cores within a batch
GF = D // N_GROUPS  # 256 features per group
HPG = N_HEADS // N_GROUPS  # 4 heads per group
PAIRS = HPG // 2  # head pairs (2 heads of 64 feats = 128 partitions)
KC = D // 128  # contraction chunks for the input projections


def build_nc(n_tok: int, loop_k: int = 1):
    """Build the single-core Bass program (same program for all 8 cores)."""
    import contextlib
    assert n_tok % 512 == 0
    QC = n_tok // 512  # query chunks of 512
    TT = n_tok // 128  # token (and key) tiles of 128

    nc = bacc.Bacc("TRN2", target_bir_lowering=False, debug=False,
                   num_devices=N_CORES)

    xqT = nc.dram_tensor("xqT", [D, n_tok], BF16, kind="ExternalInput")
    xkT = nc.dram_tensor("xkT", [D, n_tok], BF16, kind="ExternalInput")
    xvT = nc.dram_tensor("xvT", [D, n_tok], BF16, kind="ExternalInput")
    wqT = nc.dram_tensor("wqT", [D, GF], BF16, kind="ExternalInput")
    wkT = nc.dram_tensor("wkT", [D, GF], BF16, kind="ExternalInput")
    wvT = nc.dram_tensor("wvT", [D, GF], BF16, kind="ExternalInput")
    woT = nc.dram_tensor("woT", [GF, D], BF16, kind="ExternalInput")
    bq2 = nc.dram_tensor("bq2", [128, 2], F32, kind="ExternalInput")
    bk2 = nc.dram_tensor("bk2", [128, 2], F32, kind="ExternalInput")
    bvp = nc.dram_tensor("bvp", [1, GF], BF16, kind="ExternalInput")
    out_p = nc.dram_tensor("out_p", [n_tok, D], BF16, kind="ExternalOutput")
    # DRAM scratch rows for the softmax-reciprocal broadcast (DRAM source
    # APs may replicate with a 0-stride dim; SBUF partition APs may not)
    rscr = nc.dram_tensor("rscr", [4, 512], BF16, kind="Internal")

    chains = [(qc, p) for qc in range(QC) for p in range(PAIRS)]
    NCH = len(chains)
    KT2 = TT // 2  # kt2 groups per chain

    def mm(out, lhsT, rhs, **kw):
        nc.tensor.matmul(out, lhsT, rhs, **kw)

    with tile.TileContext(nc) as tc:
      with (tc.For_i(0, loop_k, 1) if loop_k > 1
            else contextlib.nullcontext()):
        with (
            tc.tile_pool(name="weights", bufs=1) as wpool,
            tc.tile_pool(name="acts", bufs=1) as apool,
            tc.tile_pool(name="xvs", bufs=1) as xvpool,
        ):
            wq_sb = wpool.tile([128, KC * GF], BF16, tag="wq")
            wk_sb = wpool.tile([128, KC * GF], BF16, tag="wk")
            wv_sb = wpool.tile([128, KC * GF], BF16, tag="wv")
            wo_sb = wpool.tile([128, 2 * D], BF16, tag="wo")
            bq_sb = wpool.tile([128, 2], F32, tag="bq")
            bk_sb = wpool.tile([128, 2], F32, tag="bk")

            # tiny bias DMAs go out on the ACT queue (also HWDGE) so their
            # DGE setup time does not delay the SP x-stream; warm-up exp
            # loads the ACT table at t~0, off the first-chain critical path
            nc.scalar.dma_start(bq_sb[:], bq2[:])
            nc.scalar.dma_start(bk_sb[:], bk2[:])
            warm_in = wpool.tile([1, 8], F32, tag="warmi")
            warm_sb = wpool.tile([1, 8], F32, tag="warmo")
            nc.vector.memset(warm_in[:], 0.0)
            nc.scalar.activation(warm_sb[:], warm_in[:], EXP)
            ones_sb = wpool.tile([128, 128], BF16, tag="ones")
            nc.vector.memset(ones_sb[:], 1.0)
            # V bias replicated across partitions via a 0-stride DRAM read
            # (bvp is already a DRAM input); folded into the V' evictions
            bvp_bc = wpool.tile([128, GF], BF16, tag="bvpbc")
            _bsrc = bvp[:]
            nc.sync.dma_start(
                bvp_bc[:],
                bass.AP(_bsrc.tensor, _bsrc.offset,
                        [[0, 128]] + list(_bsrc.ap[1:])))
            # junk matmuls keep the PE continuously busy from t~0 so the
            # P-state ramp completes before the first real projection
            junk_sb = wpool.tile([128, 512], BF16, tag="junk")
            nc.vector.memset(junk_sb[:], 0.0)

            # Q.T / K.T feature-major [2 pair-tiles x 128, n_tok]
            qt_sb = apool.tile([128, PAIRS * n_tok], BF16, tag="qt")
            kt_sb = apool.tile([128, PAIRS * n_tok], BF16, tag="kt")
            # V' token-major with per-head ones column: [n_tok, HPG*65]
            v_sb = apool.tile([128, TT * HPG * 65], BF16, tag="v")
            nc.vector.memset(v_sb[:], 1.0)

            xv_t = [xvpool.tile([128, n_tok], BF16, tag=f"xv{k}",
                                name=f"xv{k}") for k in range(KC)]

            def load_w(w_dram, w_sb, halves=1):
                kh = KC // halves
                for h in range(halves):
                    nc.sync.dma_start(
                        w_sb[:].rearrange("p (k f) -> p k f", f=GF)
                        [:, h * kh:(h + 1) * kh, :],
                        w_dram[h * kh * 128:(h + 1) * kh * 128, :]
                        .rearrange("(k p) f -> p k f", p=128))

            with tc.tile_pool(name="xqk", bufs=1) as xqkpool:
                xq_t = [xqkpool.tile([128, n_tok], BF16, tag=f"xq{k}",
                                     name=f"xq{k}") for k in range(KC)]
                xk_t = [xqkpool.tile([128, n_tok], BF16, tag=f"xk{k}",
                                     name=f"xk{k}") for k in range(KC)]

                # arrival order = need order: wq, xq, wk, xk, wv, xv, wo
                # (Q chase runs while xk streams; K's pair-0 evictions land
                # right at the xk DMA tail, gating the first exp at ~30us)
                load_w(wqT, wq_sb, halves=2)
                for k in range(KC):
                    nc.sync.dma_start(xq_t[k][:],
                                      xqT[k * 128:(k + 1) * 128, :])
                load_w(wkT, wk_sb)
                for k in range(KC):
                    nc.sync.dma_start(xk_t[k][:],
                                      xkT[k * 128:(k + 1) * 128, :])
                load_w(wvT, wv_sb)
                for k in range(KC):
                    nc.sync.dma_start(xv_t[k][:],
                                      xvT[k * 128:(k + 1) * 128, :])
                nc.sync.dma_start(
                    wo_sb[:].rearrange("p (c f) -> p c f", f=D),
                    woT[:].rearrange("(c p) f -> p c f", p=128),
                )

                # ---- Q then K projections, PSUM-chasing the arriving x
                # chunks with all 8 banks; evictions interleaved into the
                # final contraction chunk ----
                with tc.tile_pool(name="pref", bufs=8,
                                  space="PSUM") as pref:
                    warm_ps = pref.tile([128, 512], F32, tag="pref",
                                        name="warmps")
                    for _ in range(10):
                        mm(warm_ps[:], junk_sb[:, 0:128], junk_sb[:],
                           start=True, stop=True)

                    def chase(xt, w_sb, b_sb, dst_sb, pfx, order):
                        tiles = {}
                        for (m, qq) in order:
                            tiles[(m, qq)] = pref.tile(
                                [128, 512], F32, tag="pref",
                                name=f"{pfx}{m}{qq}")
                        for k in range(KC):
                            last = (k == KC - 1)
                            for i, (m, qq) in enumerate(order):
                                mm(tiles[(m, qq)][:],
                                   w_sb[:, k * GF + m * 128:
                                        k * GF + (m + 1) * 128],
                                   xt[k][:, qq * 512:(qq + 1) * 512],
                                   start=(k == 0), stop=last)
                                if last:
                                    # evictions alternate DVE/ACT so they
                                    # drain in parallel with the last mms
                                    dst = dst_sb[:, m * n_tok + qq * 512:
                                                 m * n_tok + (qq + 1) * 512]
                                    if i % 2 == 0:
                                        nc.vector.tensor_scalar_add(
                                            dst, tiles[(m, qq)][:],
                                            b_sb[:, m:m + 1])
                                    else:
                                        nc.scalar.add(
                                            dst, tiles[(m, qq)][:],
                                            b_sb[:, m:m + 1])

                    # Q first (all evicted early); K second with pair-0
                    # tiles evicted first (chain 0 reads all of pair 0)
                    chase(xq_t, wq_sb, bq_sb, qt_sb, "q",
                          [(m, qq) for qq in range(QC)
                           for m in range(PAIRS)])
                    chase(xk_t, wk_sb, bk_sb, kt_sb, "k",
                          [(m, qq) for m in range(PAIRS)
                           for qq in range(QC)])

            # ---- attention: chain-offset pipeline ----
            with (
                tc.tile_pool(name="pt", bufs=40) as ptpool,
                tc.tile_pool(name="ctx", bufs=1) as ctxpool,
                tc.tile_pool(name="craw", bufs=4) as crpool,
                tc.tile_pool(name="rcb", bufs=2) as rcpool,
                tc.tile_pool(name="ost", bufs=4) as opool,
                tc.tile_pool(name="pss", bufs=2, space="PSUM") as pss,
            ):
                pt_map = {}
                cps_map = {}
                pools = {}
                ctx_t = {}
                for qc in range(QC):
                    for p in range(PAIRS):
                        ctx_t[(p, qc)] = ctxpool.tile(
                            [128, 512], BF16, tag=f"ctx{p}{qc}",
                            name=f"ctx{p}{qc}")
                oq = []
                ost_map = {}

                def scores_group(c, j):
                    qc, p = chains[c]
                    s2 = [pss.tile([128, 1024], F32, tag="s",
                                   name=f"s{c}_{j}_{h}") for h in range(2)]
                    for half in range(2):
                        kt = 2 * j + half
                        for h in range(2):
                            mm(s2[h][:, half * 512:(half + 1) * 512],
                               kt_sb[64 * h:64 * h + 64,
                                     p * n_tok + kt * 128:
                                     p * n_tok + (kt + 1) * 128],
                               qt_sb[64 * h:64 * h + 64,
                                     p * n_tok + qc * 512:
                                     p * n_tok + (qc + 1) * 512],
                               start=True, stop=True)
                    pts = []
                    for h in range(2):
                        ptile = ptpool.tile([128, 1024], BF16, tag="pt")
                        nc.scalar.activation(ptile[:], s2[h][:], EXP,
                                             scale=1.0 / np.sqrt(DK))
                        pts.append(ptile)
                    pt_map[(c, j)] = pts

                def ctx_group(c, j):
                    qc, p = chains[c]
                    if j == 0:
                        psctx = pools["psctx"]
                        cps_map[c] = [psctx.tile([128, 512], F32, tag="cps",
                                                 name=f"cps{c}{h}")
                                      for h in range(2)]
                    pts = pt_map.pop((c, j))
                    for half in range(2):
                        kt = 2 * j + half
                        for h in range(2):
                            hh = 2 * p + h
                            mm(cps_map[c][h][0:65, :],
                               v_sb[:, kt * HPG * 65 + hh * 65:
                                       kt * HPG * 65 + (hh + 1) * 65],
                               pts[h][:, half * 512:(half + 1) * 512],
                               start=(kt == 0), stop=(kt == TT - 1))

                def norm(c):
                    qc, p = chains[c]
                    cps = cps_map.pop(c)
                    last = (c == NCH - 1)
                    for h in range(2):
                        if last:
                            # tail: read PSUM directly, skip the
                            # slot-freeing copy (nothing follows)
                            src = cps[h]
                        else:
                            src = crpool.tile([128, 512], F32, tag="craw",
                                              name="craw")
                            nc.vector.tensor_copy(src[0:65, :],
                                                  cps[h][0:65, :])
                        rc = rcpool.tile([128, 512], BF16, tag="rc",
                                         name="rc")
                        with nc.allow_low_precision(
                                reason="bf16 recip -> bcast mm"):
                            nc.vector.reciprocal(rc[64:65, :],
                                                 src[64:65, :])
                        bb = rcpool.tile([128, 512], BF16, tag="bb",
                                         name="bb")
                        if last:
                            # tail: PE is idle; matmul broadcast avoids
                            # the DMA round-trip latency
                            bp = pools["psbo"].tile([128, 512], F32,
                                                    tag="bo", name="bp")
                            mm(bp[0:64, :], ones_sb[64:65, 0:64],
                               rc[64:65, :], start=True, stop=True)
                            nc.vector.tensor_copy(bb[0:64, :], bp[0:64, :])
                        else:
                            # mid-stream: PE is saturated; replicate the
                            # reciprocal row via a DRAM bounce on the idle
                            # SP queue instead of a PE matmul (the read-
                            # back uses a 0-stride DRAM source dim)
                            row = rscr[(2 * c + h) % 4:
                                       (2 * c + h) % 4 + 1, :]
                            nc.sync.dma_start(row, rc[64:65, :])
                            brd = bass.AP(row.tensor, row.offset,
                                          [[0, 64]] + list(row.ap[1:]))
                            nc.sync.dma_start(bb[0:64, :], brd)
                        nc.vector.tensor_mul(
                            ctx_t[(p, qc)][64 * h:64 * h + 64, :],
                            src[0:64, :], bb[0:64, :])
                    if p == PAIRS - 1:
                        oq.extend((qc, s, n) for s in range(4)
                                  for n in range(2))

                ocnt = [0]

                def emit_o_unit():
                    qc, s, n = oq.pop(0)
                    key = (qc, s)
                    if n == 0:
                        ost_map[key] = opool.tile([128, D], BF16, tag="ost",
                                                  name="ost")
                    ocnt[0] += 1
                    if qc == QC - 1 and ocnt[0] % 2 == 0:
                        # tail drain: the scores PSUM banks are free after
                        # the last exp — borrow them so the O projection
                        # rotates through 4 slots instead of 2
                        op = pss.tile([128, 1024], F32, tag="s",
                                      name="opt")[:, 0:512]
                    else:
                        op = pools["psbo"].tile([128, 512], F32, tag="bo",
                                                name="op")[:, 0:512]
                    for cp in range(PAIRS):
                        mm(op,
                           ctx_t[(cp, qc)][:, s * 128:(s + 1) * 128],
                           wo_sb[:, cp * D + n * 512:cp * D + (n + 1) * 512],
                           start=(cp == 0), stop=(cp == PAIRS - 1))
                    dst = ost_map[key][:, n * 512:(n + 1) * 512]
                    if qc == QC - 1:
                        # tail: ACT is idle; alternate engines and ship each
                        # half as soon as it lands so the drain overlaps
                        if (s + n) % 2 == 0:
                            nc.scalar.copy(dst, op)
                        else:
                            nc.vector.tensor_copy(dst, op)
                    else:
                        nc.vector.tensor_copy(dst, op)
                    nc.sync.dma_start(
                        out_p[qc * 512 + s * 128:qc * 512 + (s + 1) * 128,
                              n * 512:(n + 1) * 512],
                        dst)
                    if n == 1:
                        ost_map.pop(key)

                # ---- V' construction: rounds of 4 token-tiles, one
                # accumulation region per PSUM bank (4 banks from the
                # nested pssB pool, which closes before psctx/psbo open) --
                NR = TT // 4  # 4 rounds
                vslots = {}

                def v_mms(r, k):
                    if k == 0:
                        vslots[r] = [pools["pssB"].tile(
                            [128, 512], F32, tag="vp", name=f"vsl{r}{i}")
                            for i in range(4)]
                    for q in range(4):
                        tt = 4 * r + q
                        reg = vslots[r][q][:, 0:256]
                        mm(reg,
                           xv_t[k][:, tt * 128:(tt + 1) * 128],
                           wv_sb[:, k * GF:(k + 1) * GF],
                           start=(k == 0), stop=(k == KC - 1))
                    if k == KC - 1:
                        for q in range(4):
                            tt = 4 * r + q
                            reg = vslots[r][q][:, 0:256]
                            dst = v_sb[:, tt * HPG * 65:(tt + 1) * HPG * 65]
                            nc.vector.tensor_add(
                                dst.rearrange("p (h c) -> p h c",
                                              c=65)[:, :, 0:DK],
                                reg.rearrange("p (h c) -> p h c", c=DK),
                                bvp_bc[:].rearrange("p (h c) -> p h c",
                                                    c=DK))
                        del vslots[r]

                with tc.tile_pool(name="pssB", bufs=4,
                                  space="PSUM") as pssB:
                    pools["pssB"] = pssB
                    # scores chain 0, V' round 0 arrival-aligned: the
                    # exp-paced score emission stays ahead of the xv
                    # arrivals, so v-unit (0, j) never blocks the queue
                    for j in range(KT2):
                        scores_group(0, j)
                        v_mms(0, j)
                    # scores chain 1 with the remaining V' rounds packed
                    # into the ACT-paced slack (all xv chunks resident)
                    vq = [(r, k) for r in range(1, NR) for k in range(KC)]
                    for j in range(KT2):
                        scores_group(1, j)
                        for _ in range(3):
                            if vq:
                                v_mms(*vq.pop(0))

                with (
                    tc.tile_pool(name="psctx", bufs=2,
                                 space="PSUM") as psctx,
                    tc.tile_pool(name="psbo", bufs=2, space="PSUM") as psbo,
                ):
                    pools["psctx"] = psctx
                    pools["psbo"] = psbo
                    # drain the two buffered chains while ACT chews the
                    # freshly emitted scores of chains 2 and 3
                    for j in range(KT2):
                        scores_group(2, j)
                        ctx_group(0, j)
                    norm(0)
                    for j in range(KT2):
                        scores_group(3, j)
                        ctx_group(1, j)
                    norm(1)
                    # remaining scores emitted from a global queue: the
                    # ctx-only re-convergence loop takes them at half rate
                    # so ACT never starves at the chain-4 boundary
                    sq = [(cc, jj) for cc in range(4, NCH)
                          for jj in range(KT2)]
                    for j in range(KT2):
                        if j % 2 == 0 and sq:
                            scores_group(*sq.pop(0))
                        ctx_group(2, j)
                        for _ in range(2):
                            if oq:
                                emit_o_unit()
                    norm(2)
                    for c in range(3, NCH):
                        for j in range(KT2):
                            if sq:
                                scores_group(*sq.pop(0))
                            ctx_group(c, j)
                            if oq:
                                emit_o_unit()
                        norm(c)
                    while oq:
                        emit_o_unit()

    nc.compile()
    return nc


_NC_CACHE: dict[int, object] = {}


def get_nc(n_tok: int):
    if n_tok not in _NC_CACHE:
        _NC_CACHE[n_tok] = build_nc(n_tok)
    return _NC_CACHE[n_tok]


def make_in_maps(query, key, value, w_q, b_q, w_k, b_k, w_v, b_v, w_o, b_o):
    n_tok = query.shape[1]
    bf16 = ml_dtypes.bfloat16
    xT = {}
    for b in range(B):
        xT[("q", b)] = np.ascontiguousarray(query[b].T.astype(bf16))
        xT[("k", b)] = np.ascontiguousarray(key[b].T.astype(bf16))
        xT[("v", b)] = np.ascontiguousarray(value[b].T.astype(bf16))
    in_maps = []
    for core in range(N_CORES):
        b, g = divmod(core, N_GROUPS)
        gs = slice(g * GF, (g + 1) * GF)
        in_maps.append({
            "xqT": xT[("q", b)],
            "xkT": xT[("k", b)],
            "xvT": xT[("v", b)],
            "wqT": np.ascontiguousarray(w_q[gs, :].T.astype(bf16)),
            "wkT": np.ascontiguousarray(w_k[gs, :].T.astype(bf16)),
            "wvT": np.ascontiguousarray(w_v[gs, :].T.astype(bf16)),
            "woT": np.ascontiguousarray(w_o[:, gs].T.astype(bf16)),
            "bq2": np.ascontiguousarray(
                b_q[gs].reshape(2, 128).T, np.float32),
            "bk2": np.ascontiguousarray(
                b_k[gs].reshape(2, 128).T, np.float32),
            "bvp": np.ascontiguousarray(b_v[gs].reshape(1, GF).astype(bf16)),
        })
    return in_maps


def kernel(**inputs):
    query = np.asarray(inputs["query"], np.float32)
    n_tok = query.shape[1]
    nc = get_nc(n_tok)
    in_maps = make_in_maps(
        query, np.asarray(inputs["key"], np.float32),
        np.asarray(inputs["value"], np.float32),
        np.asarray(inputs["w_q"], np.float32), np.asarray(inputs["b_q"], np.float32),
        np.asarray(inputs["w_k"], np.float32), np.asarray(inputs["b_k"], np.float32),
        np.asarray(inputs["w_v"], np.float32), np.asarray(inputs["b_v"], np.float32),
        np.asarray(inputs["w_o"], np.float32), np.asarray(inputs["b_o"], np.float32),
    )
    res = run_bass_kernel_spmd(nc, in_maps, core_ids=list(range(N_CORES)))
    out = np.zeros((B, n_tok, D), np.float32)
    for core in range(N_CORES):
        b = core // N_GROUPS
        out[b] += res.results[core]["out_p"].astype(np.float32)
    out += np.asarray(inputs["b_o"], np.float32)
    return out


# revision 45
# speedup vs baseline: 2.5163x; 2.0729x over previous
"""Multi-head attention (B=2, N=2048, D=1024, H=16) sharded over 8 trn2 cores.

Sharding: batch (2) x head-groups (4 groups of 4 heads) = 8 cores.
Each core computes, for its (batch b, head-group g):
  Q.T/K.T feature-major and V token-major projections of its group,
  S.T = K @ Q.T scores (keys on partitions, queries on free axis),
  P.T = exp(S.T / 8)  (no max subtraction -- scores are ~N(0,1), safe in fp32),
  ctx'.T = [V | ones].T @ P.T  (ones column yields softmax denominators),
  ctx.T normalized via DMA-broadcast 1/sums (DRAM bounce, 0-stride read),
  partial O = ctx.T.T @ w_o_g.T  (row-parallel O projection).
Host sums the 4 group partials per batch and adds b_o.

Schedule: input DMA order xq -> xk -> xv so Q/K projections finish first
(PSUM-chasing the arriving chunks with all 8 banks), scores+exp start
~30us while xv still streams; V' is built in the xv window using the
O-projection PSUM slots, with its bias folded into the evictions via a
0-stride DRAM broadcast of b_v; ctx for chain c consumes exp outputs
produced a full chain earlier (pt pool buffers ~2 chains) so the PE never
waits on the ACT engine latency; mid-stream softmax normalization runs on
DVE+SP-DMA only (the PE matmul broadcast is used only on the idle tail);
the final O drain borrows the free scores-PSUM banks for a 4-slot rotation.

Matmul operands are bf16 (host pre-converts inputs; fp32 PSUM accumulation);
biases are applied in fp32 during PSUM eviction.
"""

import os
import sys

for _p in ("/opt/trn_rl_repo",):
    if _p not in sys.path and os.path.isdir(_p):
        sys.path.insert(0, _p)

import ml_dtypes
import numpy as np

import concourse.bass as bass
import concourse.tile as tile
from concourse import bacc, mybir
from concourse.bass_utils import run_bass_kernel_spmd

F32 = mybir.dt.float32
BF16 = mybir.dt.bfloat16
EXP = mybir.ActivationFunctionType.Exp

B = 2
D = 1024
N_HEADS = 16
DK = 64
N_CORES = 8
N_GROUPS = 4  # head groups (4 heads each) across cores within a batch
GF = D // N_GROUPS  # 256 features per group
HPG = N_HEADS // N_GROUPS  # 4 heads per group
PAIRS = HPG // 2  # head pairs (2 heads of 64 feats = 128 partitions)
KC = D // 128  # contraction chunks for the input projections


def build_nc(n_tok: int, loop_k: int = 1):
    """Build the single-core Bass program (same program for all 8 cores)."""
    import contextlib
    assert n_tok % 512 == 0
    QC = n_tok // 512  # query chunks of 512
    TT = n_tok // 128  # token (and key) tiles of 128

    nc = bacc.Bacc("TRN2", target_bir_lowering=False, debug=False,
                   num_devices=N_CORES)

    xqT = nc.dram_tensor("xqT", [D, n_tok], BF16, kind="ExternalInput")
    xkT = nc.dram_tensor("xkT", [D, n_tok], BF16, kind="ExternalInput")
    xvT = nc.dram_tensor("xvT", [D, n_tok], BF16, kind="ExternalInput")
    wqT = nc.dram_tensor("wqT", [D, GF], BF16, kind="ExternalInput")
    wkT = nc.dram_tensor("wkT", [D, GF], BF16, kind="ExternalInput")
    wvT = nc.dram_tensor("wvT", [D, GF], BF16, kind="ExternalInput")
    woT = nc.dram_tensor("woT", [GF, D], BF16, kind="ExternalInput")
    bq2 = nc.dram_tensor("bq2", [128, 2], F32, kind="ExternalInput")
    bk2 = nc.dram_tensor("bk2", [128, 2], F32, kind="ExternalInput")
    bvp = nc.dram_tensor("bvp", [1, GF], BF16, kind="ExternalInput")
    out_p = nc.dram_tensor("out_p", [n_tok, D], BF16, kind="ExternalOutput")
    # DRAM scratch rows for the softmax-reciprocal broadcast (DRAM source
    # APs may replicate with a 0-stride dim; SBUF partition APs may not)
    rscr = nc.dram_tensor("rscr", [4, 512], BF16, kind="Internal")

    chains = [(qc, p) for qc in range(QC) for p in range(PAIRS)]
    NCH = len(chains)
    KT2 = TT // 2  # kt2 groups per chain

    def mm(out, lhsT, rhs, **kw):
        nc.tensor.matmul(out, lhsT, rhs, **kw)

    with tile.TileContext(nc) as tc:
      with (tc.For_i(0, loop_k, 1) if loop_k > 1
            else contextlib.nullcontext()):
        with (
            tc.tile_pool(name="weights", bufs=1) as wpool,
            tc.tile_pool(name="acts", bufs=1) as apool,
            tc.tile_pool(name="xvs", bufs=1) as xvpool,
        ):
            wq_sb = wpool.tile([128, KC * GF], BF16, tag="wq")
            wk_sb = wpool.tile([128, KC * GF], BF16, tag="wk")
            wv_sb = wpool.tile([128, KC * GF], BF16, tag="wv")
            wo_sb = wpool.tile([128, 2 * D], BF16, tag="wo")
            bq_sb = wpool.tile([128, 2], F32, tag="bq")
            bk_sb = wpool.tile([128, 2], F32, tag="bk")

            # tiny bias DMAs go out on the ACT queue (also HWDGE) so their
            # DGE setup time does not delay the SP x-stream; warm-up exp
            # loads the ACT table at t~0, off the first-chain critical path
            nc.scalar.dma_start(bq_sb[:], bq2[:])
            nc.scalar.dma_start(bk_sb[:], bk2[:])
            warm_in = wpool.tile([1, 8], F32, tag="warmi")
            warm_sb = wpool.tile([1, 8], F32, tag="warmo")
            nc.vector.memset(warm_in[:], 0.0)
            nc.scalar.activation(warm_sb[:], warm_in[:], EXP)
            ones_sb = wpool.tile([128, 128], BF16, tag="ones")
            nc.vector.memset(ones_sb[:], 1.0)
            # V bias replicated across partitions via a 0-stride DRAM read
            # (bvp is already a DRAM input); folded into the V' evictions
            bvp_bc = wpool.tile([128, GF], BF16, tag="bvpbc")
            _bsrc = bvp[:]
            nc.sync.dma_start(
                bvp_bc[:],
                bass.AP(_bsrc.tensor, _bsrc.offset,
                        [[0, 128]] + list(_bsrc.ap[1:])))
            # junk matmuls keep the PE continuously busy from t~0 so the
            # P-state ramp completes before the first real projection
            junk_sb = wpool.tile([128, 512], BF16, tag="junk")
            nc.vector.memset(junk_sb[:], 0.0)

            # Q.T / K.T feature-major [2 pair-tiles x 128, n_tok]
            qt_sb = apool.tile([128, PAIRS * n_tok], BF16, tag="qt")
            kt_sb = apool.tile([128, PAIRS * n_tok], BF16, tag="kt")
            # V' token-major with per-head ones column: [n_tok, HPG*65]
            v_sb = apool.tile([128, TT * HPG * 65], BF16, tag="v")
            nc.vector.memset(v_sb[:], 1.0)

            xv_t = [xvpool.tile([128, n_tok], BF16, tag=f"xv{k}",
                                name=f"xv{k}") for k in range(KC)]

            def load_w(w_dram, w_sb, halves=1):
                kh = KC // halves
                for h in range(halves):
                    nc.sync.dma_start(
                        w_sb[:].rearrange("p (k f) -> p k f", f=GF)
                        [:, h * kh:(h + 1) * kh, :],
                        w_dram[h * kh * 128:(h + 1) * kh * 128, :]
                        .rearrange("(k p) f -> p k f", p=128))

            with tc.tile_pool(name="xqk", bufs=1) as xqkpool:
                xq_t = [xqkpool.tile([128, n_tok], BF16, tag=f"xq{k}",
                                     name=f"xq{k}") for k in range(KC)]
                xk_t = [xqkpool.tile([128, n_tok], BF16, tag=f"xk{k}",
                                     name=f"xk{k}") for k in range(KC)]

                # arrival order = need order: wq, xq, wk, xk, wv, xv, wo
                # (Q chase runs while xk streams; K's pair-0 evictions land
                # right at the xk DMA tail, gating the first exp at ~30us)
                load_w(wqT, wq_sb, halves=2)
                for k in range(KC):
                    nc.sync.dma_start(xq_t[k][:],
                                      xqT[k * 128:(k + 1) * 128, :])
                load_w(wkT, wk_sb)
                for k in range(KC):
                    nc.sync.dma_start(xk_t[k][:],
                                      xkT[k * 128:(k + 1) * 128, :])
                load_w(wvT, wv_sb)
                for k in range(KC):
                    nc.sync.dma_start(xv_t[k][:],
                                      xvT[k * 128:(k + 1) * 128, :])
                nc.sync.dma_start(
                    wo_sb[:].rearrange("p (c f) -> p c f", f=D),
                    woT[:].rearrange("(c p) f -> p c f", p=128),
                )

                # ---- Q then K projections, PSUM-chasing the arriving x
                # chunks with all 8 banks; evictions interleaved into the
                # final contraction chunk ----
                with tc.tile_pool(name="pref", bufs=8,
                                  space="PSUM") as pref:
                    warm_ps = pref.tile([128, 512], F32, tag="pref",
                                        name="warmps")
                    for _ in range(10):
                        mm(warm_ps[:], junk_sb[:, 0:128], junk_sb[:],
                           start=True, stop=True)

                    def chase(xt, w_sb, b_sb, dst_sb, pfx, order):
                        tiles = {}
                        for (m, qq) in order:
                            tiles[(m, qq)] = pref.tile(
                                [128, 512], F32, tag="pref",
                                name=f"{pfx}{m}{qq}")
                        for k in range(KC):
                            last = (k == KC - 1)
                            for i, (m, qq) in enumerate(order):
                                mm(tiles[(m, qq)][:],
                                   w_sb[:, k * GF + m * 128:
                                        k * GF + (m + 1) * 128],
                                   xt[k][:, qq * 512:(qq + 1) * 512],
                                   start=(k == 0), stop=last)
                                if last:
                                    # evictions alternate DVE/ACT so they
                                    # drain in parallel with the last mms
                                    dst = dst_sb[:, m * n_tok + qq * 512:
                                                 m * n_tok + (qq + 1) * 512]
                                    if i % 2 == 0:
                                        nc.vector.tensor_scalar_add(
                                            dst, tiles[(m, qq)][:],
                                            b_sb[:, m:m + 1])
                                    else:
                                        nc.scalar.add(
                                            dst, tiles[(m, qq)][:],
                                            b_sb[:, m:m + 1])

                    # Q first (all evicted early); K second with pair-0
                    # tiles evicted first (chain 0 reads all of pair 0)
                    chase(xq_t, wq_sb, bq_sb, qt_sb, "q",
                          [(m, qq) for qq in range(QC)
                           for m in range(PAIRS)])
                    chase(xk_t, wk_sb, bk_sb, kt_sb, "k",
                          [(m, qq) for m in range(PAIRS)
                           for qq in range(QC)])

            # ---- attention: chain-offset pipeline ----
            with (
                tc.tile_pool(name="pt", bufs=40) as ptpool,
                tc.tile_pool(name="ctx", bufs=1) as ctxpool,
                tc.tile_pool(name="craw", bufs=4) as crpool,
                tc.tile_pool(name="rcb", bufs=2) as rcpool,
                tc.tile_pool(name="ost", bufs=4) as opool,
                tc.tile_pool(name="pss", bufs=2, space="PSUM") as pss,
            ):
                pt_map = {}
                cps_map = {}
                pools = {}
                ctx_t = {}
                for qc in range(QC):
                    for p in range(PAIRS):
                        ctx_t[(p, qc)] = ctxpool.tile(
                            [128, 512], BF16, tag=f"ctx{p}{qc}",
                            name=f"ctx{p}{qc}")
                oq = []
                ost_map = {}

                def scores_group(c, j):
                    qc, p = chains[c]
                    s2 = [pss.tile([128, 1024], F32, tag="s",
                                   name=f"s{c}_{j}_{h}") for h in range(2)]
                    for half in range(2):
                        kt = 2 * j + half
                        for h in range(2):
                            mm(s2[h][:, half * 512:(half + 1) * 512],
                               kt_sb[64 * h:64 * h + 64,
                                     p * n_tok + kt * 128:
                                     p * n_tok + (kt + 1) * 128],
                               qt_sb[64 * h:64 * h + 64,
                                     p * n_tok + qc * 512:
                                     p * n_tok + (qc + 1) * 512],
                               start=True, stop=True)
                    pts = []
                    for h in range(2):
                        ptile = ptpool.tile([128, 1024], BF16, tag="pt")
                        nc.scalar.activation(ptile[:], s2[h][:], EXP,
                                             scale=1.0 / np.sqrt(DK))
                        pts.append(ptile)
                    pt_map[(c, j)] = pts

                def ctx_group(c, j):
                    qc, p = chains[c]
                    if j == 0:
                        psctx = pools["psctx"]
                        cps_map[c] = [psctx.tile([128, 512], F32, tag="cps",
                                                 name=f"cps{c}{h}")
                                      for h in range(2)]
                    pts = pt_map.pop((c, j))
                    for half in range(2):
                        kt = 2 * j + half
                        for h in range(2):
                            hh = 2 * p + h
                            mm(cps_map[c][h][0:65, :],
                               v_sb[:, kt * HPG * 65 + hh * 65:
                                       kt * HPG * 65 + (hh + 1) * 65],
                               pts[h][:, half * 512:(half + 1) * 512],
                               start=(kt == 0), stop=(kt == TT - 1))

                def norm(c):
                    qc, p = chains[c]
                    cps = cps_map.pop(c)
                    last = (c == NCH - 1)
                    for h in range(2):
                        if last:
                            # tail: read PSUM directly, skip the
                            # slot-freeing copy (nothing follows)
                            src = cps[h]
                        else:
                            src = crpool.tile([128, 512], F32, tag="craw",
                                              name="craw")
                            nc.vector.tensor_copy(src[0:65, :],
                                                  cps[h][0:65, :])
                        rc = rcpool.tile([128, 512], BF16, tag="rc",
                                         name="rc")
                        with nc.allow_low_precision(
                                reason="bf16 recip -> bcast mm"):
                            nc.vector.reciprocal(rc[64:65, :],
                                                 src[64:65, :])
                        bb = rcpool.tile([128, 512], BF16, tag="bb",
                                         name="bb")
                        if last:
                            # tail: PE is idle; matmul broadcast avoids
                            # the DMA round-trip latency
                            bp = pools["psbo"].tile([128, 512], F32,
                                                    tag="bo", name="bp")
                            mm(bp[0:64, :], ones_sb[64:65, 0:64],
                               rc[64:65, :], start=True, stop=True)
                            nc.vector.tensor_copy(bb[0:64, :], bp[0:64, :])
                        else:
                            # mid-stream: PE is saturated; replicate the
                            # reciprocal row via a DRAM bounce on the idle
                            # SP queue instead of a PE matmul (the read-
                            # back uses a 0-stride DRAM source dim)
                            row = rscr[(2 * c + h) % 4:
                                       (2 * c + h) % 4 + 1, :]
                            nc.sync.dma_start(row, rc[64:65, :])
                            brd = bass.AP(row.tensor, row.offset,
                                          [[0, 64]] + list(row.ap[1:]))
                            nc.sync.dma_start(bb[0:64, :], brd)
                        nc.vector.tensor_mul(
                            ctx_t[(p, qc)][64 * h:64 * h + 64, :],
                            src[0:64, :], bb[0:64, :])
                    if p == PAIRS - 1:
                        oq.extend((qc, s, n) for s in range(4)
                                  for n in range(2))

                ocnt = [0]

                def emit_o_unit():
                    qc, s, n = oq.pop(0)
                    key = (qc, s)
                    if n == 0:
                        ost_map[key] = opool.tile([128, D], BF16, tag="ost",
                                                  name="ost")
                    ocnt[0] += 1
                    if qc == QC - 1 and ocnt[0] % 2 == 0:
                        # tail drain: the scores PSUM banks are free after
                        # the last exp — borrow them so the O projection
                        # rotates through 4 slots instead of 2
                        op = pss.tile([128, 1024], F32, tag="s",
                                      name="opt")[:, 0:512]
                    else:
                        op = pools["psbo"].tile([128, 512], F32, tag="bo",
                                                name="op")[:, 0:512]
                    for cp in range(PAIRS):
                        mm(op,
                           ctx_t[(cp, qc)][:, s * 128:(s + 1) * 128],
                           wo_sb[:, cp * D + n * 512:cp * D + (n + 1) * 512],
                           start=(cp == 0), stop=(cp == PAIRS - 1))
                    dst = ost_map[key][:, n * 512:(n + 1) * 512]
                    if qc == QC - 1:
                        # tail: ACT is idle; alternate engines and ship each
                        # half as soon as it lands so the drain overlaps
                        if (s + n) % 2 == 0:
                            nc.scalar.copy(dst, op)
                        else:
                            nc.vector.tensor_copy(dst, op)
                    else:
                        nc.vector.tensor_copy(dst, op)
                    nc.sync.dma_start(
                        out_p[qc * 512 + s * 128:qc * 512 + (s + 1) * 128,
                              n * 512:(n + 1) * 512],
                        dst)
                    if n == 1:
                        ost_map.pop(key)

                # ---- V' construction: rounds of 4 token-tiles, one
                # accumulation region per PSUM bank (4 banks from the
                # nested pssB pool, which closes before psctx/psbo open) --
                NR = TT // 4  # 4 rounds
                vslots = {}

                def v_mms(r, k):
                    if k == 0:
                        vslots[r] = [pools["pssB"].tile(
                            [128, 512], F32, tag="vp", name=f"vsl{r}{i}")
                            for i in range(4)]
                    for q in range(4):
                        tt = 4 * r + q
                        reg = vslots[r][q][:, 0:256]
                        mm(reg,
                           xv_t[k][:, tt * 128:(tt + 1) * 128],
                           wv_sb[:, k * GF:(k + 1) * GF],
                           start=(k == 0), stop=(k == KC - 1))
                    if k == KC - 1:
                        for q in range(4):
                            tt = 4 * r + q
                            reg = vslots[r][q][:, 0:256]
                            dst = v_sb[:, tt * HPG * 65:(tt + 1) * HPG * 65]
                            nc.vector.tensor_add(
                                dst.rearrange("p (h c) -> p h c",
                                              c=65)[:, :, 0:DK],
                                reg.rearrange("p (h c) -> p h c", c=DK),
                                bvp_bc[:].rearrange("p (h c) -> p h c",
                                                    c=DK))
                        del vslots[r]

                with tc.tile_pool(name="pssB", bufs=4,
                                  space="PSUM") as pssB:
                    pools["pssB"] = pssB
                    # scores chain 0, V' round 0 arrival-aligned: the
                    # exp-paced score emission stays ahead of the xv
                    # arrivals, so v-unit (0, j) never blocks the queue
                    for j in range(KT2):
                        scores_group(0, j)
                        v_mms(0, j)
                    # scores chain 1 with the remaining V' rounds packed
                    # into the ACT-paced slack (all xv chunks resident)
                    vq = [(r, k) for r in range(1, NR) for k in range(KC)]
                    for j in range(KT2):
                        scores_group(1, j)
                        for _ in range(3):
                            if vq:
                                v_mms(*vq.pop(0))

                with (
                    tc.tile_pool(name="psctx", bufs=2,
                                 space="PSUM") as psctx,
                    tc.tile_pool(name="psbo", bufs=2, space="PSUM") as psbo,
                ):
                    pools["psctx"] = psctx
                    pools["psbo"] = psbo
                    # drain the two buffered chains while ACT chews the
                    # freshly emitted scores of chains 2 and 3
                    for j in range(KT2):
                        scores_group(2, j)
                        ctx_group(0, j)
                    norm(0)
                    for j in range(KT2):
                        scores_group(3, j)
                        ctx_group(1, j)
                    norm(1)
                    # remaining scores emitted from a global queue: the
                    # ctx-only re-convergence loop takes them at half rate
                    # so ACT never starves at the chain-4 boundary
                    sq = [(cc, jj) for cc in range(4, NCH)
                          for jj in range(KT2)]
                    for j in range(KT2):
                        if j % 2 == 0 and sq:
                            scores_group(*sq.pop(0))
                        ctx_group(2, j)
                        for _ in range(2):
                            if oq:
                                emit_o_unit()
                    norm(2)
                    for c in range(3, NCH):
                        for j in range(KT2):
                            if sq:
                                scores_group(*sq.pop(0))
                            ctx_group(c, j)
                            if oq:
                                emit_o_unit()
                        norm(c)
                    while oq:
                        emit_o_unit()

    nc.compile()
    return nc


_NC_CACHE: dict[int, object] = {}


def get_nc(n_tok: int):
    if n_tok not in _NC_CACHE:
        _NC_CACHE[n_tok] = build_nc(n_tok)
    return _NC_CACHE[n_tok]


def make_in_maps(query, key, value, w_q, b_q, w_k, b_k, w_v, b_v, w_o, b_o):
    n_tok = query.shape[1]
    bf16 = ml_dtypes.bfloat16
    xT = {}
    for b in range(B):
        xT[("q", b)] = np.ascontiguousarray(query[b].T.astype(bf16))
        xT[("k", b)] = np.ascontiguousarray(key[b].T.astype(bf16))
        xT[("v", b)] = np.ascontiguousarray(value[b].T.astype(bf16))
    in_maps = []
    for core in range(N_CORES):
        b, g = divmod(core, N_GROUPS)
        gs = slice(g * GF, (g + 1) * GF)
        in_maps.append({
            "xqT": xT[("q", b)],
            "xkT": xT[("k", b)],
            "xvT": xT[("v", b)],
            "wqT": np.ascontiguousarray(w_q[gs, :].T.astype(bf16)),
            "wkT": np.ascontiguousarray(w_k[gs, :].T.astype(bf16)),
            "wvT": np.ascontiguousarray(w_v[gs, :].T.astype(bf16)),
            "woT": np.ascontiguousarray(w_o[:, gs].T.astype(bf16)),
            "bq2": np.ascontiguousarray(
                b_q[gs].reshape(2, 128).T, np.float32),
            "bk2": np.ascontiguousarray(
                b_k[gs].reshape(2, 128).T, np.float32),
            "bvp": np.ascontiguousarray(b_v[gs].reshape(1, GF).astype(bf16)),
        })
    return in_maps


def kernel(**inputs):
    query = np.asarray(inputs["query"], np.float32)
    n_tok = query.shape[1]
    nc = get_nc(n_tok)
    in_maps = make_in_maps(
        query, np.asarray(inputs["key"], np.float32),
        np.asarray(inputs["value"], np.float32),
        np.asarray(inputs["w_q"], np.float32), np.asarray(inputs["b_q"], np.float32),
        np.asarray(inputs["w_k"], np.float32), np.asarray(inputs["b_k"], np.float32),
        np.asarray(inputs["w_v"], np.float32), np.asarray(inputs["b_v"], np.float32),
        np.asarray(inputs["w_o"], np.float32), np.asarray(inputs["b_o"], np.float32),
    )
    res = run_bass_kernel_spmd(nc, in_maps, core_ids=list(range(N_CORES)))
    out = np.zeros((B, n_tok, D), np.float32)
    for core in range(N_CORES):
        b = core // N_GROUPS
        out[b] += res.results[core]["out_p"].astype(np.float32)
    out += np.asarray(inputs["b_o"], np.float32)
    return out
